# revision 8
# baseline (speedup 1.0000x reference)
"""Trainium2 Bass kernel for nn_MemristorArray (B=128, I=512, O=512).

Math (see reference):
  low = poly(poly_low, x); high = poly(poly_high, x); d = high - low
  out[b,o] = sum_i low[b,i] + (d @ r)[b,o]
           + sum_i noise[i,o] * sqrt(g2[b,i] * |low[b,i] + d[b,i]*r[i,o]|)
  with g2[b,i] = 4*KBT*BW/(|x|+eps) + 2*e*BW.

Key restructuring: for fixed (b,i), f(rho) = sqrt(g2*|low + d*rho|) is a
scalar function of rho = r[i,o] in [0,1]; an L2 fit in rho turns the noise
term into matmuls sum_k A_k @ (noise o r^k). The output is dominated by the
coherent sum_i low bias (rms ~380) while the noise term is ~1e-5 relative,
so K=0 (one alpha_0 @ noise slice) suffices, and fp16 (10-bit mantissa)
suffices for the main d @ r slice: total ~1.3e-4 norm rel err, ~5e-3 max
elementwise.

Device kernel: both 512-row slices stack into one 1024-row fp16 contraction
[d.T; 256*alpha0.T] x [r; noise/256]. Sharding is 4 contraction groups x 2
output halves across 8 cores: each core runs two [128c,128b,256f] fp16
matmuls into one f32 PSUM tile. Inputs arrive as one packed
[128 x (U0 V0 U1 V1)] fp16 tile (1536B/partition lines) split over both
HWDGE queues by partition range; PSUM is copied to SBUF in halves on ACT
and DVE and DMA'd out as a [128,256] f32 partial. Host sums the 4 partials
per output half (the unshard step of this contraction sharding) and adds
the exact sum_i low bias.
"""
import numpy as np
from contextlib import ExitStack

import concourse.bass as bass
import concourse.tile as tile
from concourse import bacc, mybir
from concourse.bass_utils import run_bass_kernel_spmd

B, I, O = 128, 512, 512
NCORES = 8
G = 4                      # contraction groups (1024 stacked rows / 256)
H = 2                      # output-dim halves
OW = O // H                # 256 output cols per core
CHUNKS = 2                 # 128-row contraction chunks per core
W = 128 + OW               # packed cols per chunk: stationary then moving

f32 = mybir.dt.float32
f16 = mybir.dt.float16

BW = 1e-08
KBT = 1.380649e-23 * 300.0
EPS = 1e-12
C1_J = 4.0 * KBT * BW
C2_S = 2.0 * float(np.e) * BW

NFIT = 64                  # rho samples for the K=0 L2 fit (mean over [0,1])
ASC = 256.0                # alpha0 scale-up / noise scale-down (fp16 range)

PROFILE = False
TRACE_KW = {}
LAST_RESULTS = None

_BUILT = None
_NOISE = None


def _build():
    nc = bacc.Bacc("TRN2", target_bir_lowering=False, debug=False)
    pk_d = nc.dram_tensor("pk", [128, CHUNKS * W], f16, kind="ExternalInput")
    out_d = nc.dram_tensor("out", [128, OW], f32, kind="ExternalOutput")

    with tile.TileContext(nc) as tc, ExitStack() as ctx:
        pool = ctx.enter_context(tc.tile_pool(name="s", bufs=1))
        pp = ctx.enter_context(tc.tile_pool(name="ps", bufs=1, space="PSUM"))

        pk = pool.tile([128, CHUNKS * W], f16)
        # Chunk-major split: each queue first delivers its half of chunk 0 so
        # the first matmul can start while chunk 1 is still streaming.
        nc.sync.dma_start(out=pk[:64, :W], in_=pk_d.ap()[:64, :W])
        nc.scalar.dma_start(out=pk[64:, :W], in_=pk_d.ap()[64:, :W])
        nc.sync.dma_start(out=pk[:64, W:], in_=pk_d.ap()[:64, W:])
        nc.scalar.dma_start(out=pk[64:, W:], in_=pk_d.ap()[64:, W:])

        acc = pp.tile([128, OW], f32)
        for c in range(CHUNKS):
            nc.tensor.matmul(acc,
                             pk[:, c * W:c * W + 128],
                             pk[:, c * W + 128:(c + 1) * W],
                             start=(c == 0), stop=(c == CHUNKS - 1))

        # Single full-width DVE copy (readers of one PSUM tile serialize
        # anyway), then both out-DMAs issue in parallel on the two queues.
        h = OW // 2
        outsb = pool.tile([128, OW], f32)
        nc.vector.tensor_scalar_mul(outsb, acc, 1.0)
        nc.scalar.dma_start(out=out_d.ap()[:, :h], in_=outsb[:, :h])
        nc.sync.dma_start(out=out_d.ap()[:, h:], in_=outsb[:, h:])

    nc.compile()
    return nc


def _get_noise():
    # Reproduce the reference's fixed noise draw on the same default backend
    # the reference would use; fall back to CPU if that fails.
    import jax
    import jax.numpy as jnp
    try:
        n = np.asarray(jax.random.normal(jax.random.key(42), (I, O),
                                         dtype=jnp.float32))
    except Exception:
        f = jax.jit(lambda: jax.random.normal(jax.random.key(42), (I, O),
                                              dtype=jnp.float32), backend="cpu")
        n = np.asarray(f())
    return n


def kernel(inputs, poly_low, poly_high, r):
    global _BUILT, _NOISE, LAST_RESULTS
    if _BUILT is None:
        _BUILT = _build()
    if _NOISE is None:
        _NOISE = _get_noise()

    x = inputs.astype(np.float64)
    pl = poly_low.astype(np.float64)
    ph = poly_high.astype(np.float64)
    rr = r.astype(np.float64)
    low = np.polynomial.polynomial.polyval(x, pl)
    high = np.polynomial.polynomial.polyval(x, ph)
    d = high - low
    g2 = C1_J / (np.abs(x) + EPS) + C2_S

    # K=0 noise fit: alpha0(b,i) = mean over rho in [0,1] of f(rho)
    rho = (np.arange(NFIT) + 0.5) / NFIT
    a0 = np.sqrt(g2[:, :, None]
                 * np.abs(low[:, :, None] + d[:, :, None] * rho[None, None])
                 ).mean(axis=2)

    # Stacked [1024, 128] stationary (contraction-major) and [1024, 512]
    # moving fp16 slices: main d @ r plus the rescaled noise slice.
    ustack = np.concatenate([d.T, (a0 * ASC).T], axis=0).astype(np.float16)
    vstack = np.concatenate([rr, _NOISE / ASC], axis=0).astype(np.float16)

    in_maps = []
    for k in range(NCORES):
        g, h = divmod(k, H)
        parts = []
        for c in range(CHUNKS):
            rb = slice(g * 256 + c * 128, g * 256 + (c + 1) * 128)
            parts.append(ustack[rb])
            parts.append(vstack[rb, h * OW:(h + 1) * OW])
        in_maps.append(dict(pk=np.ascontiguousarray(
            np.concatenate(parts, axis=1))))

    res = run_bass_kernel_spmd(_BUILT, in_maps, core_ids=list(range(NCORES)),
                               trace=PROFILE, **TRACE_KW)
    LAST_RESULTS = res

    out = np.zeros((B, O), dtype=np.float64)
    for k in range(NCORES):
        g, h = divmod(k, H)
        out[:, h * OW:(h + 1) * OW] += res.results[k]["out"].astype(np.float64)
    out += low.sum(axis=1)[:, None]
    return np.ascontiguousarray(out.astype(np.float32))


# revision 10
# speedup vs baseline: 1.0125x; 1.0125x over previous
"""Trainium2 Bass kernel for nn_MemristorArray (B=128, I=512, O=512).

Math (see reference):
  low = poly(poly_low, x); high = poly(poly_high, x); d = high - low
  out[b,o] = sum_i low[b,i] + (d @ r)[b,o]
           + sum_i noise[i,o] * sqrt(g2[b,i] * |low[b,i] + d[b,i]*r[i,o]|)
  with g2[b,i] = 4*KBT*BW/(|x|+eps) + 2*e*BW.

Key restructuring: for fixed (b,i), f(rho) = sqrt(g2*|low + d*rho|) is a
scalar function of rho = r[i,o] in [0,1]; an L2 fit in rho turns the noise
term into matmuls sum_k A_k @ (noise o r^k). The output is dominated by the
coherent sum_i low bias (rms ~380) while the noise term is ~1e-5 relative,
so K=0 (one alpha_0 @ noise slice) suffices, and fp16 (10-bit mantissa)
suffices for the main d @ r slice: total ~1.3e-4 norm rel err, ~5e-3 max
elementwise.

Device kernel: both 512-row slices stack into one 1024-row fp16 contraction
[d.T; 256*alpha0.T] x [r; noise/256]. Sharding is 4 contraction groups x 2
output halves across 8 cores: each core runs two [128c,128b,256f] fp16
matmuls into one f32 PSUM tile. Inputs arrive as one packed
[128 x (U0 V0 U1 V1)] fp16 tile (1536B/partition lines) split over both
HWDGE queues by partition range; PSUM is copied to SBUF in halves on ACT
and DVE and DMA'd out as a [128,256] f32 partial. Host sums the 4 partials
per output half (the unshard step of this contraction sharding) and adds
the exact sum_i low bias.
"""
import numpy as np
from contextlib import ExitStack

import concourse.bass as bass
import concourse.tile as tile
from concourse import bacc, mybir
from concourse.bass_utils import run_bass_kernel_spmd

B, I, O = 128, 512, 512
NCORES = 8
G = 4                      # contraction groups (1024 stacked rows / 256)
H = 2                      # output-dim halves
OW = O // H                # 256 output cols per core
CHUNKS = 2                 # 128-row contraction chunks per core
W = 128 + OW               # packed cols per chunk: stationary then moving

f32 = mybir.dt.float32
f16 = mybir.dt.float16

BW = 1e-08
KBT = 1.380649e-23 * 300.0
EPS = 1e-12
C1_J = 4.0 * KBT * BW
C2_S = 2.0 * float(np.e) * BW

NFIT = 64                  # rho samples for the K=0 L2 fit (mean over [0,1])
ASC = 256.0                # alpha0 scale-up / noise scale-down (fp16 range)

PROFILE = False
TRACE_KW = {}
LAST_RESULTS = None

_BUILT = None
_NOISE = None


def _build():
    nc = bacc.Bacc("TRN2", target_bir_lowering=False, debug=False)
    pk_d = nc.dram_tensor("pk", [128, CHUNKS * W], f16, kind="ExternalInput")
    out_d = nc.dram_tensor("out", [128, OW], f32, kind="ExternalOutput")

    with tile.TileContext(nc) as tc, ExitStack() as ctx:
        pool = ctx.enter_context(tc.tile_pool(name="s", bufs=1))
        pp = ctx.enter_context(tc.tile_pool(name="ps", bufs=1, space="PSUM"))

        pk = pool.tile([128, CHUNKS * W], f16)
        # Chunk-major split: each queue first delivers its half of chunk 0 so
        # the first matmul can start while chunk 1 is still streaming.
        nc.sync.dma_start(out=pk[:64, :W], in_=pk_d.ap()[:64, :W])
        nc.scalar.dma_start(out=pk[64:, :W], in_=pk_d.ap()[64:, :W])
        nc.sync.dma_start(out=pk[:64, W:], in_=pk_d.ap()[:64, W:])
        nc.scalar.dma_start(out=pk[64:, W:], in_=pk_d.ap()[64:, W:])

        acc = pp.tile([128, OW], f32)
        for c in range(CHUNKS):
            nc.tensor.matmul(acc,
                             pk[:, c * W:c * W + 128],
                             pk[:, c * W + 128:(c + 1) * W],
                             start=(c == 0), stop=(c == CHUNKS - 1))

        # Copy halves on ACT and DVE, each followed by its own queue's
        # out-DMA (ACT's DMA issues on the same engine, no semaphore hop).
        h = OW // 2
        out_a = pool.tile([128, h], f32)
        out_b = pool.tile([128, h], f32)
        nc.vector.tensor_scalar_mul(out_b, acc[:, h:], 1.0)
        nc.scalar.copy(out_a, acc[:, :h])
        nc.scalar.dma_start(out=out_d.ap()[:, :h], in_=out_a)
        nc.sync.dma_start(out=out_d.ap()[:, h:], in_=out_b)

    nc.compile()
    return nc


def _get_noise():
    # Reproduce the reference's fixed noise draw on the same default backend
    # the reference would use; fall back to CPU if that fails.
    import jax
    import jax.numpy as jnp
    try:
        n = np.asarray(jax.random.normal(jax.random.key(42), (I, O),
                                         dtype=jnp.float32))
    except Exception:
        f = jax.jit(lambda: jax.random.normal(jax.random.key(42), (I, O),
                                              dtype=jnp.float32), backend="cpu")
        n = np.asarray(f())
    return n


def kernel(inputs, poly_low, poly_high, r):
    global _BUILT, _NOISE, LAST_RESULTS
    if _BUILT is None:
        _BUILT = _build()
    if _NOISE is None:
        _NOISE = _get_noise()

    x = inputs.astype(np.float64)
    pl = poly_low.astype(np.float64)
    ph = poly_high.astype(np.float64)
    rr = r.astype(np.float64)
    low = np.polynomial.polynomial.polyval(x, pl)
    high = np.polynomial.polynomial.polyval(x, ph)
    d = high - low
    g2 = C1_J / (np.abs(x) + EPS) + C2_S

    # K=0 noise fit: alpha0(b,i) = mean over rho in [0,1] of f(rho)
    rho = (np.arange(NFIT) + 0.5) / NFIT
    a0 = np.sqrt(g2[:, :, None]
                 * np.abs(low[:, :, None] + d[:, :, None] * rho[None, None])
                 ).mean(axis=2)

    # Stacked [1024, 128] stationary (contraction-major) and [1024, 512]
    # moving fp16 slices: main d @ r plus the rescaled noise slice.
    ustack = np.concatenate([d.T, (a0 * ASC).T], axis=0).astype(np.float16)
    vstack = np.concatenate([rr, _NOISE / ASC], axis=0).astype(np.float16)

    in_maps = []
    for k in range(NCORES):
        g, h = divmod(k, H)
        parts = []
        for c in range(CHUNKS):
            rb = slice(g * 256 + c * 128, g * 256 + (c + 1) * 128)
            parts.append(ustack[rb])
            parts.append(vstack[rb, h * OW:(h + 1) * OW])
        in_maps.append(dict(pk=np.ascontiguousarray(
            np.concatenate(parts, axis=1))))

    res = run_bass_kernel_spmd(_BUILT, in_maps, core_ids=list(range(NCORES)),
                               trace=PROFILE, **TRACE_KW)
    LAST_RESULTS = res

    out = np.zeros((B, O), dtype=np.float64)
    for k in range(NCORES):
        g, h = divmod(k, H)
        out[:, h * OW:(h + 1) * OW] += res.results[k]["out"].astype(np.float64)
    out += low.sum(axis=1)[:, None]
    return np.ascontiguousarray(out.astype(np.float32))
